# revision 10
# baseline (speedup 1.0000x reference)
"""DMP network kernel for Trainium2 (8 NeuronCores, pure data parallel).

Math: the reference is a 54->54 linear layer followed by a 301-step Euler
integration of a DMP (dynamic movement primitive). The phase variable xp and
hence the RBF activations psi are batch-independent, and the (y, z) scan is a
linear time-invariant recurrence driven by fx = (g - y0) * (w @ P_t). The
whole scan collapses to the closed form

    Y[b, d, t] = a_t * y0 + beta_t * g + (g - y0) * (w @ Q)[b, d, t]

with a, beta [T] and Q [N, T] computed on the host from c / sigma2 in float64.

Scaling a batch row of x by a per-row scalar commutes through any matmul, so
(g - y0) * (w @ Q) = (x_ext * dcol) @ (W2w.T @ Q) with x_ext = [x, 1] and
dcol = g - y0. The device pipeline per 128-row batch tile (x arrives
host-transposed as xT [55, batch] in fp16, duplicated on partitions
0..54 / 64..118, with ones planted at partitions 55,56 and 119,120):

  1. HBC matmul (per 4 tiles): hb [128, 512] = ch.T @ xT, where ch's columns
     replicate the dcol coefficient across partitions 0..54 (and 64..118 for
     DOF 1) and put the y0/g coefficients at partitions 55,56 / 119,120.
  2. One VectorE multiply: mt [121, 128] = xin * hb (fp16 out).
  3. One matmul per DOF: Y_d [128, 302] = mt[d].T @ [A_d; a; beta] -- fp16
     operands, f32 PSUM. The two matmuls land on PE row groups 0..63 and
     64..127 (tile_position row tiling) so they overlap in the array.
  4. PSUM->SBUF fp16-casting copies (split ScalarE/VectorE) into a grouped
     output tile [128, 4, 604]; one contiguous DMA per 4 tiles. The host
     re-interleaves the grouped fp16 layout and upcasts to f32.

All I/O is fp16: output bytes halve (the kernel is HBM-write-bound), and the
closed-form trajectory is smooth O(1) data, so fp16 keeps the relative error
around 1e-3.
"""

import os
import numpy as np

# -- problem constants (fixed by the reference) -------------------------------
N = 25
DOF = 2
TAU = 3.0
DT = 0.01
A_X = 2.0
A_Z = 48.0
B_Z = A_Z / 4.0
T = 301
D_IN = 54           # DOF * (N + 2)
B = 65536
N_CORES = 8
B_CORE = B // N_CORES          # 8192
P = 128                        # batch rows per tile
N_TILES = B_CORE // P          # 64
X_CHUNK = 8                    # tiles per input DMA
HB_CHUNK = 4                   # tiles per head-broadcast matmul
Y_CHUNK = 2                    # tiles per output DMA
X_ROWS = 57                    # input rows shipped from the host
N_WARM = 8                     # PE warm-up matmuls to raise the pstate
D_PAD = 55                     # 54 features + ones row
T_PAD = 302                    # even moving-dim count; col 301 is a zero pad
W_HI = 64                      # partition offset of the DOF-1 block
MT_H = 121                     # mt rows: 0..56 d0 block, 64..120 d1 block
F_OUT = DOF * T_PAD            # 604 output cols per batch row (2 pad cols)
N_GROUPS = N_TILES // Y_CHUNK  # 16 output DMA groups


# -- host-side closed-form constants ------------------------------------------
def _closed_form_consts(c, sigma2):
    """a [T], beta [T], Q [N, T] in float64."""
    c = np.asarray(c, np.float64)
    sigma2 = np.asarray(sigma2, np.float64)
    alpha = DT / TAU

    xp = np.empty(T)
    xp[0] = 1.0
    for t in range(T - 1):
        xp[t + 1] = xp[t] - (A_X * xp[t] / TAU) * DT
    psi = np.exp(-0.5 * (xp[:, None] - c[None, :]) ** 2 / sigma2[None, :])  # [T, N]
    S = psi.sum(1)
    Pmat = (psi * (xp / S)[:, None]).T                                      # [N, T]

    A = np.array([[1.0, alpha], [-alpha * A_Z * B_Z, 1.0 - alpha * A_Z]])
    a = np.empty(T)
    bvec = np.empty(T)
    M = np.eye(2)
    for t in range(T):
        a[t] = M[0, 0]
        bvec[t] = M[0, 1]
        M = A @ M
    beta = A_Z * B_Z * alpha * np.concatenate([[0.0], np.cumsum(bvec)[:-1]])

    H = np.zeros((T, T))
    for t in range(1, T):
        H[:t, t] = alpha * bvec[t - 1::-1]
    Q = Pmat @ H                                                            # [N, T]
    return a, beta, Q


def _host_inputs(x, W, b, c, sigma2, scale):
    """Build per-core input maps (numpy, fp16 device payloads)."""
    a, beta, Q = _closed_form_consts(c, sigma2)

    W2 = np.asarray(W, np.float64) * np.asarray(scale, np.float64)[:, None]
    b2 = np.asarray(b, np.float64) * np.asarray(scale, np.float64)

    # w2e[:, j] = 55-vector [W2[j, :], b2[j]] -- the ones row carries the bias
    w2e = np.concatenate([W2.T, b2[None, :]], axis=0)       # [55, 54]

    # head-broadcast coefficients ch [55, 128]
    ch = np.zeros((D_PAD, P), np.float64)
    for d, lo in ((0, 0), (1, W_HI)):
        base = d * (N + 2)
        dc = w2e[:, base + 1] - w2e[:, base]
        ch[:, lo:lo + D_PAD] = dc[:, None]
        ch[:, lo + D_PAD] = w2e[:, base]          # y0_d coeff
        ch[:, lo + D_PAD + 1] = w2e[:, base + 1]  # g_d coeff
    ch = np.ascontiguousarray(ch.astype(np.float16))

    # Y-matmul coefficients cy [128, 604]: rows 0..56 d0, rows 64..120 d1
    cy = np.zeros((P, DOF * T_PAD), np.float64)
    for d, lo in ((0, 0), (1, W_HI)):
        base = d * (N + 2)
        cy[lo:lo + D_PAD, d * T_PAD:d * T_PAD + T] = w2e[:, base + 2:base + 2 + N] @ Q
        cy[lo + D_PAD, d * T_PAD:d * T_PAD + T] = a
        cy[lo + D_PAD + 1, d * T_PAD:d * T_PAD + T] = beta
    cy = np.ascontiguousarray(cy.astype(np.float16))

    # host-transposed x image [57, B] fp16: x on rows 0..53, bias-ones row 54,
    # head pass-through ones rows 55,56. The device duplicates rows 0..56 onto
    # partitions 64..120 (DOF-1 block) with a GpSimd copy; partitions 57..63
    # stay garbage -- they only ever multiply against hb rows that are zero.
    xT = np.zeros((D_PAD + 2, B), np.float16)
    xT[:D_IN] = np.asarray(x, np.float16).T
    xT[D_IN] = 1.0
    xT[D_PAD:D_PAD + 2] = 1.0

    in_maps = []
    for ci in range(N_CORES):
        in_maps.append({
            "x": np.ascontiguousarray(xT[:, ci * B_CORE:(ci + 1) * B_CORE]),
            "ch": ch,
            "cy": cy,
        })
    return in_maps


# -- bass program --------------------------------------------------------------
_NC_CACHE = None


def _build_program():
    global _NC_CACHE
    if _NC_CACHE is not None:
        return _NC_CACHE

    import concourse.bacc as bacc
    import concourse.tile as tile
    from concourse import mybir
    from contextlib import ExitStack

    f32 = mybir.dt.float32
    f16 = mybir.dt.float16
    u32 = mybir.dt.uint32

    nc = bacc.Bacc(
        "TRN2",
        target_bir_lowering=False,
        debug=False,
        num_devices=N_CORES,
    )
    x_d = nc.declare_dram_parameter("x", [X_ROWS, B_CORE], f16, isOutput=False)
    ch_d = nc.declare_dram_parameter("ch", [D_PAD, P], f16, isOutput=False)
    cy_d = nc.declare_dram_parameter("cy", [P, DOF * T_PAD], f16, isOutput=False)
    # grouped output: group g holds tiles 4g..4g+3 as [128, 4, 604] fp16
    y_d = nc.declare_dram_parameter("y", [N_GROUPS * P, Y_CHUNK * F_OUT], f16,
                                    isOutput=True)

    with tile.TileContext(nc) as tc, ExitStack() as ctx:
        consts = ctx.enter_context(tc.tile_pool(name="consts", bufs=1))
        xin_p = ctx.enter_context(tc.tile_pool(name="xin", bufs=4))
        mt_p = ctx.enter_context(tc.tile_pool(name="mt", bufs=8))
        yout_p = ctx.enter_context(tc.tile_pool(name="yout", bufs=4))
        hb_p = ctx.enter_context(tc.tile_pool(name="hb", bufs=2, space="PSUM"))
        ps_p = ctx.enter_context(tc.tile_pool(name="ps", bufs=3, space="PSUM"))

        ch_sb = consts.tile([D_PAD, P], f16)
        nc.sync.dma_start(ch_sb[:], ch_d[:])
        cy_sb = consts.tile([P, DOF * T_PAD], f16)
        nc.sync.dma_start(cy_sb[:], cy_d[:])

        # PE warm-up: the Tensor engine starts at the 0.65/1.2 GHz pstates and
        # only reaches 2.4 GHz after ~3us of continuous execution. Burn the
        # DMA-latency dead time at program start on dummy matmuls over an
        # SBUF scratch tile so real matmuls issue against a hot PE.
        warm_sb = consts.tile([W_HI, 512], f16)
        nc.gpsimd.memset(warm_sb[:], 0)
        for _ in range(N_WARM):
            wps = hb_p.tile([P, 512], f32, tag="hb")
            nc.tensor.matmul(wps[:], warm_sb[0:D_PAD, 0:P], warm_sb[0:D_PAD, :],
                             start=True, stop=True)

        ysb = None
        mt4 = None
        for ci in range(N_TILES // X_CHUNK):
            CW = X_CHUNK * P
            xin = xin_p.tile([P, CW], f16)
            src = x_d[:, ci * CW:(ci + 1) * CW]
            # ScalarE HWDGE queue: separate FIFO from the output DMAs.
            # First chunk lands in halves so tile 0 starts sooner; the DOF-1
            # partition block is duplicated on GpSimd, bitcast to u32 to halve
            # the element count (GpSimd is element-rate-bound).
            H = CW // 2 if ci == 0 else CW
            for c0 in range(0, CW, H):
                nc.scalar.dma_start(xin[0:X_ROWS, c0:c0 + H], src[:, c0:c0 + H])
                nc.gpsimd.tensor_copy(
                    xin[W_HI:W_HI + X_ROWS, c0:c0 + H].bitcast(u32),
                    xin[0:X_ROWS, c0:c0 + H].bitcast(u32))

            for j in range(X_CHUNK):
                i = ci * X_CHUNK + j
                jc = j * P

                if j % HB_CHUNK == 0:
                    HW_ = HB_CHUNK * P
                    hb = hb_p.tile([P, HW_], f32, tag="hb")
                    nc.tensor.matmul(hb[:], ch_sb[:], xin[0:D_PAD, jc:jc + HW_],
                                     start=True, stop=True)
                    # mt rows: [x*dcol0 (55); y0_0; g_0; 0..; x*dcol1; y0_1; g_1]
                    # one batched multiply per 4 tiles amortizes DVE overhead
                    mt4 = mt_p.tile([MT_H, HW_], f16, tag="mt")
                    nc.vector.tensor_mul(mt4[:], xin[0:MT_H, jc:jc + HW_],
                                         hb[0:MT_H, :])
                hcol = (j % HB_CHUNK) * P

                # two-bank PSUM tile: d0 in bank 0, d1 in bank 1, so one cast
                # instruction later reads both via a bank-strided 3D AP
                ps = ps_p.tile([P, 2, 512], f32, tag="ps")
                nc.tensor.matmul(ps[:, 0, 0:T_PAD], mt4[0:D_PAD + 2, hcol:hcol + P],
                                 cy_sb[0:D_PAD + 2, 0:T_PAD],
                                 start=True, stop=True)
                nc.tensor.matmul(ps[:, 1, 0:T_PAD], mt4[W_HI:MT_H, hcol:hcol + P],
                                 cy_sb[W_HI:MT_H, T_PAD:2 * T_PAD],
                                 start=True, stop=True)

                if j % Y_CHUNK == 0:
                    ysb = yout_p.tile([P, Y_CHUNK * F_OUT], f16)
                oc = (j % Y_CHUNK) * F_OUT
                dst = ysb[:, oc:oc + 2 * T_PAD]
                # cast rotation: ScalarE ~678ns vs VectorE ~630ns per paired
                # cast, and VectorE also carries the mt multiply -> DVE 2 : ACT 3
                if i % 5 < 2:
                    nc.vector.tensor_copy(dst, ps[:, :, 0:T_PAD])
                else:
                    nc.scalar.copy(dst, ps[:, :, 0:T_PAD])

                if j % Y_CHUNK == Y_CHUNK - 1:
                    g = i // Y_CHUNK
                    nc.sync.dma_start(y_d[g * P:(g + 1) * P, :], ysb[:])

    nc.compile()
    _NC_CACHE = nc
    return nc


_LAST_RESULTS = None


def kernel(x, W, b, c, sigma2, scale):
    global _LAST_RESULTS
    from concourse.bass_utils import run_bass_kernel_spmd

    assert x.shape == (B, D_IN), x.shape
    nc = _build_program()
    in_maps = _host_inputs(x, W, b, c, sigma2, scale)
    res = run_bass_kernel_spmd(nc, in_maps, list(range(N_CORES)))
    _LAST_RESULTS = res

    out = np.empty((B, DOF * T), np.float32)
    for ci in range(N_CORES):
        yg = np.asarray(res.results[ci]["y"])            # [16*128, 4*604] fp16
        yg = yg.reshape(N_GROUPS, P, Y_CHUNK, F_OUT)
        yg = yg.transpose(0, 2, 1, 3).reshape(B_CORE, F_OUT)
        dst = out[ci * B_CORE:(ci + 1) * B_CORE]
        dst[:, 0:T] = yg[:, 0:T]
        dst[:, T:2 * T] = yg[:, T_PAD:T_PAD + T]
    return out
